# revision 1
# baseline (speedup 1.0000x reference)
"""Trainium2 Bass kernel for ConvAttnPool.

Model (per batch row b):
  e   = W_emb[x[b]]                       # [T=2500, E=100]
  h   = tanh(conv1d(e, conv_w, pad=5))    # [T'=2501, F=50]
  s   = h @ U_w.T                         # [T', Y]  (scores, transposed layout)
  a   = softmax(s, axis=t)
  y   = sum_f final_w[y,f] * (a.T @ h)[y,f] + final_b[y]

Device strategy: data-parallel over batch (8 cores x 1 row). Per core:
  - indirect-DMA gather of embedding rows, PE-transpose into eT [E, T_pad]
  - conv as 10 accumulated matmuls -> hT [F, T'] (+ duplicate copy on
    partitions 64..113 so score matmuls can row-pack two 64-row tiles)
  - scores S^T [t, y] via paired matmuls (tile_position (0,0)/(64,0)),
    exp directly from PSUM on the scalar engine (bottleneck: ~22M exps)
  - num[f,y] (+ denominator via ones column appended to h) via matmul
    h_ext[t,51].T @ exps[t,y] accumulated over t-chunks
  - numer = colsum(num * final_w.T) via ones-vector matmul; y = numer/denom + b
"""

import sys

import numpy as np

if "/opt/trn_rl_repo" not in sys.path:
    sys.path.insert(0, "/opt/trn_rl_repo")

import concourse.bass as bass
import concourse.tile as tile
from concourse import bacc, mybir
from concourse.bass_utils import run_bass_kernel_spmd
from concourse.masks import make_identity

VOCAB, E, F, KW, Y = 51917, 100, 50, 10, 8921
B, T = 8, 2500
PAD = 5
TP = T + 1          # conv output length 2501
TC = 20             # t-chunks of 128 (covers 2560)
TPADDED = TC * 128  # 2560
TLAST = TP - 19 * 128  # valid rows in last t-chunk = 69
GLAST = T - 19 * 128   # valid rows in last gather chunk = 68
ET_W = 2570            # conv reads up to 2048+9+512; zero-padded tail
YTILE = 512
NYT = 18            # y tiles of 512 (padded 9216)
YPAD = NYT * YTILE  # 9216
CONV_NT = 5         # conv t'-tiles of 512 (last = 453)
HEXTW = 66          # h_ext chunk width: 50 h cols, zeros, ones col at DEN (even M)
DEN = 64            # denominator column / partition (32-aligned for engine APs)
FP32 = mybir.dt.float32
I32 = mybir.dt.int32

# number of 128-t-chunks handled by one PSUM mega tile / one exp activation
EXP_GROUPS = [(0, 3), (3, 3), (6, 3), (9, 3), (12, 3), (15, 3), (18, 2)]


def build_program(mm_kind="fp32", repeat=1):
    """mm_kind: 'fp32' (4 cyc/row), 'f32r' (1 cyc/row at N>=256, reduced
    precision), matmul operand dtype for the big matmuls."""
    nc = bacc.Bacc(
        "TRN2",
        target_bir_lowering=False,
        debug=False,
        num_devices=B,
    )

    if mm_kind == "f32r":
        MF = mybir.dt.float32r
    elif mm_kind == "fp32":
        MF = FP32
    else:
        raise ValueError(mm_kind)

    x_d = nc.dram_tensor("x_idx", [128, TC], I32, kind="ExternalInput")
    emb_d = nc.dram_tensor("w_emb", [VOCAB, E], FP32, kind="ExternalInput")
    cwt_d = nc.dram_tensor("cwt", [E, KW * F], MF, kind="ExternalInput")
    cb_d = nc.dram_tensor("conv_b", [F, 1], FP32, kind="ExternalInput")
    uw_d = nc.dram_tensor("uw2", [128, YPAD], MF, kind="ExternalInput")
    fwt_d = nc.dram_tensor("fwt", [F, YPAD], FP32, kind="ExternalInput")
    fb_d = nc.dram_tensor("fb", [NYT, YTILE], FP32, kind="ExternalInput")
    y_d = nc.dram_tensor("y", [NYT, YTILE], FP32, kind="ExternalOutput")

    EXP = mybir.ActivationFunctionType.Exp
    TANH = mybir.ActivationFunctionType.Tanh

    U32 = mybir.dt.uint32
    ONE_BITS = 0x3F800000

    def ms0(ap):
        nc.vector.memset(ap.bitcast(U32), 0)

    def ms1(ap):
        nc.vector.memset(ap.bitcast(U32), ONE_BITS)

    with tile.TileContext(nc) as tc:
      for _rep in range(repeat):
        with tc.tile_pool(name="singles", bufs=1) as singles:
            identity = singles.tile([128, 128], FP32)
            make_identity(nc, identity[:])
            ones_col = singles.tile([F, 2], MF)
            ms1(ones_col[:])
            cb_sb = singles.tile([F, 1], FP32)
            nc.sync.dma_start(cb_sb[:], cb_d[:])
            cwt_sb = singles.tile([E, KW * F], MF)
            nc.sync.dma_start(cwt_sb[:], cwt_d[:])
            x_sb = singles.tile([128, TC], I32)
            nc.sync.dma_start(x_sb[:], x_d[:])
            fb_sb = singles.tile([NYT, YTILE], FP32)
            nc.sync.dma_start(fb_sb[:], fb_d[:])

            eT = singles.tile([E, ET_W], MF)
            ms0(eT[:])
            hT = singles.tile([128, TPADDED], FP32)
            nc.vector.memset(hT[:], 0.0)
            hT_r = singles.tile([128, TPADDED], MF)
            h_ext = singles.tile([128, TC * HEXTW], MF)
            ms0(h_ext[:])
            numer_all = singles.tile([NYT, YTILE], FP32)
            denom_all = singles.tile([NYT, YTILE], FP32)

            # ---------------- setup: gather + conv + h_ext ----------------
            with (
                tc.tile_pool(name="gat", bufs=3) as gat,
                tc.tile_pool(name="ps_setup", bufs=2, space="PSUM") as ps_setup,
            ):
                # embedding gather: one offset per partition per chunk
                # (HW indirect DMA honors a single row offset per partition)
                for c in range(TC):
                    rows = 128 if c < TC - 1 else GLAST
                    er = gat.tile([128, E], FP32, tag="er")
                    nc.gpsimd.indirect_dma_start(
                        out=er[0:rows, :],
                        out_offset=None,
                        in_=emb_d[:, :],
                        in_offset=bass.IndirectOffsetOnAxis(
                            ap=x_sb[0:rows, c : c + 1], axis=0
                        ),
                    )
                    pt = ps_setup.tile([128, 128], FP32, tag="ps")
                    nc.tensor.transpose(
                        pt[0:E, 0:rows], er[0:rows, :], identity[0:rows, 0:rows]
                    )
                    nc.vector.tensor_copy(
                        eT[:, PAD + 128 * c : PAD + 128 * c + rows], pt[0:E, 0:rows]
                    )

                # conv1d: hT[f, t'] = sum_k cwt[:, k].T @ eT[:, t'+k]
                for j in range(CONV_NT):
                    pc = ps_setup.tile([128, 512], FP32, tag="psc")
                    for k in range(KW):
                        nc.tensor.matmul(
                            pc[0:F, :],
                            cwt_sb[:, k * F : (k + 1) * F],
                            eT[:, j * YTILE + k : j * YTILE + k + YTILE],
                            start=(k == 0),
                            stop=(k == KW - 1),
                        )
                    nc.scalar.activation(
                        hT[0:F, j * YTILE : (j + 1) * YTILE],
                        pc[0:F, :],
                        TANH,
                        bias=cb_sb[:],
                    )
                    # duplicate onto partitions 64.. for row-packed matmuls
                    nc.sync.dma_start(
                        hT[64 : 64 + F, j * YTILE : (j + 1) * YTILE],
                        hT[0:F, j * YTILE : (j + 1) * YTILE],
                    )
                    # rounded copy used as the score-matmul stationary operand
                    nc.vector.tensor_copy(
                        hT_r[:, j * YTILE : (j + 1) * YTILE],
                        hT[:, j * YTILE : (j + 1) * YTILE],
                    )

                # h_ext[t, c*51 : c*51+50] = h chunk, col 50 = ones
                for c in range(TC):
                    tv = 128 if c < TC - 1 else TLAST
                    ph = ps_setup.tile([128, 128], FP32, tag="ps")
                    nc.tensor.transpose(
                        ph[0:tv, 0:F],
                        hT[0:F, c * 128 : c * 128 + tv],
                        identity[0:F, 0:F],
                    )
                    nc.vector.tensor_copy(
                        h_ext[0:tv, c * HEXTW : c * HEXTW + F], ph[0:tv, 0:F]
                    )
                    ms1(h_ext[0:tv, c * HEXTW + DEN : c * HEXTW + DEN + 1])

            # ---------------- main loop over y tiles ----------------
            with (
                tc.tile_pool(name="uwp", bufs=2) as uwp,
                tc.tile_pool(name="fwp", bufs=2) as fwp,
                tc.tile_pool(name="expp", bufs=2) as expp,
                tc.tile_pool(name="finp", bufs=2) as finp,
                tc.tile_pool(name="ps_s", bufs=2, space="PSUM") as ps_s,
                tc.tile_pool(name="ps_acc", bufs=1, space="PSUM") as ps_acc,
                tc.tile_pool(name="ps_cs", bufs=1, space="PSUM") as ps_cs,
            ):
                for yt in range(NYT):
                    ys = yt * YTILE
                    uw_t = uwp.tile([128, YTILE], MF, tag="uw")
                    nc.sync.dma_start(uw_t[:], uw_d[:, ys : ys + YTILE])
                    fw_t = fwp.tile([F, YTILE], FP32, tag="fw")
                    nc.sync.dma_start(fw_t[:], fwt_d[:, ys : ys + YTILE])

                    exps = expp.tile([128, TC * YTILE], MF, tag="exps")
                    acc = ps_acc.tile([128, YTILE], FP32, tag="acc")

                    for c0, ng in EXP_GROUPS:
                        ps = ps_s.tile([128, 3 * YTILE], FP32, tag="s")
                        # paired score matmuls on row groups (0,0) / (64,0)
                        for i in range(ng):
                            c = c0 + i
                            base = 0 if (c % 2 == 0) else 64
                            nc.tensor.matmul(
                                ps[:, i * YTILE : (i + 1) * YTILE],
                                hT_r[base : base + 64, c * 128 : (c + 1) * 128],
                                uw_t[base : base + 64, :],
                                start=True,
                                stop=True,
                                tile_position=(base, 0),
                            )
                        nc.scalar.activation(
                            exps[:, c0 * YTILE : (c0 + ng) * YTILE],
                            ps[:, 0 : ng * YTILE],
                            EXP,
                        )
                        # pooling matmuls: acc[m, y] += h_ext[t, m]^T exps[t, y]
                        for i in range(ng):
                            c = c0 + i
                            nc.tensor.matmul(
                                acc[0:HEXTW, :],
                                h_ext[:, c * HEXTW : (c + 1) * HEXTW],
                                exps[:, c * YTILE : (c + 1) * YTILE],
                                start=(c == 0),
                                stop=(c == TC - 1),
                            )

                    # finalize tile: numer = ones.T @ (num * fwT), denom = acc[50]
                    p_t = finp.tile([F, YTILE], MF, tag="p")
                    nc.vector.tensor_mul(p_t[:], acc[0:F, :], fw_t[:])
                    strip = finp.tile([128, YTILE], FP32, tag="strip")
                    nc.vector.tensor_copy(strip[DEN : DEN + 1, :], acc[DEN : DEN + 1, :])
                    cs = ps_cs.tile([2, YTILE], FP32, tag="cs")
                    nc.tensor.matmul(cs[:], ones_col[:], p_t[:], start=True, stop=True)
                    nc.vector.tensor_copy(strip[0:1, :], cs[0:1, :])
                    nc.sync.dma_start(numer_all[yt : yt + 1, :], strip[0:1, :])
                    nc.sync.dma_start(denom_all[yt : yt + 1, :], strip[DEN : DEN + 1, :])

                # epilogue: y = numer / denom + fb
                recip = singles.tile([NYT, YTILE], FP32)
                nc.vector.reciprocal(recip[:], denom_all[:])
                yv = singles.tile([NYT, YTILE], FP32)
                nc.vector.tensor_mul(yv[:], numer_all[:], recip[:])
                yout = singles.tile([NYT, YTILE], FP32)
                nc.vector.tensor_add(yout[:], yv[:], fb_sb[:])
                nc.sync.dma_start(y_d[:, :], yout[:])

    nc.compile()
    return nc


_CACHE = {}

MM_KIND = "f32r"


def get_program(mm_kind=None, repeat=1):
    if mm_kind is None:
        mm_kind = MM_KIND
    key = (mm_kind, repeat)
    if key not in _CACHE:
        _CACHE[key] = build_program(mm_kind, repeat)
    return _CACHE[key]


def make_in_maps(x, W_emb, conv_w, conv_b, U_w, final_w, final_b):
    x = np.asarray(x).astype(np.int32)
    x_pad = np.zeros((B, TPADDED), np.int32)
    x_pad[:, :T] = x
    # x_maps[b][p, c] = x[b, c*128 + p]
    x_maps = np.ascontiguousarray(x_pad.reshape(B, TC, 128).transpose(0, 2, 1))

    emb = np.ascontiguousarray(np.asarray(W_emb, np.float32))
    # cwt[e, k*F + f] = conv_w[f, e, k]
    cwt = np.ascontiguousarray(
        np.asarray(conv_w, np.float32).transpose(1, 2, 0).reshape(E, KW * F)
    )
    cb = np.ascontiguousarray(np.asarray(conv_b, np.float32).reshape(F, 1))

    uwT = np.asarray(U_w, np.float32).T  # [F, Y]
    uw2 = np.zeros((128, YPAD), np.float32)
    uw2[0:F, :Y] = uwT
    uw2[64 : 64 + F, :Y] = uwT

    fwt = np.zeros((F, YPAD), np.float32)
    fwt[:, :Y] = np.asarray(final_w, np.float32).T

    fb = np.zeros((NYT, YTILE), np.float32)
    fb.reshape(-1)[:Y] = np.asarray(final_b, np.float32)

    return [
        dict(
            x_idx=x_maps[b],
            w_emb=emb,
            cwt=cwt,
            conv_b=cb,
            uw2=uw2,
            fwt=fwt,
            fb=fb,
        )
        for b in range(B)
    ]


def run(in_maps, trace=False, **kwargs):
    nc = get_program()
    return run_bass_kernel_spmd(
        nc, in_maps, core_ids=list(range(B)), trace=trace, **kwargs
    )


def kernel(x, W_emb, conv_w, conv_b, U_w, final_w, final_b):
    in_maps = make_in_maps(x, W_emb, conv_w, conv_b, U_w, final_w, final_b)
    res = run(in_maps)
    out = np.stack(
        [res.results[b]["y"].reshape(-1)[:Y] for b in range(B)]
    ).astype(np.float32)
    return out



# revision 5
# speedup vs baseline: 1.0558x; 1.0558x over previous
"""Trainium2 Bass kernel for ConvAttnPool.

Model (per batch row b):
  e   = W_emb[x[b]]                       # [T=2500, E=100]
  h   = tanh(conv1d(e, conv_w, pad=5))    # [T'=2501, F=50]
  s   = h @ U_w.T                         # [T', Y]  (scores, transposed layout)
  a   = softmax(s, axis=t)
  y   = sum_f final_w[y,f] * (a.T @ h)[y,f] + final_b[y]

Device strategy: data-parallel over batch (8 cores x 1 row). Per core:
  - indirect-DMA gather of embedding rows, PE-transpose into eT [E, T_pad]
  - conv as 10 accumulated f32r matmuls -> hT [F, T'] (tanh on ACT)
  - scores via fp8 DoubleRow matmuls: stationary hT_r2 [25, 2, 128] per
    t-chunk (fp8 copy of h split into two 25-row K-tiles, placed by DMA),
    moving uw8 [25, 2, W]; one matmul per (chunk, y-tile) at 0.5 cyc/col
  - exp on ACT from PSUM score groups (4+3-bank double buffering) into
    bf16 exps [128, 20, W]; ACT is the bottleneck (~22M exps/core)
  - pooling f32r matmuls h_ext[t, 68].T @ exps -> acc[68, W] over 20 chunks
    (col 64 of h_ext is ones -> denominator row)
  - finalize: numer = ones.T @ (acc[0:50] * fwT); cs matmul shares the acc
    PSUM bank via tag rotation; y = numer/denom + fb
Last y-tile is 217 wide (Y = 17*512 + 217) to skip padded exp columns.
"""

import sys

import numpy as np

if "/opt/trn_rl_repo" not in sys.path:
    sys.path.insert(0, "/opt/trn_rl_repo")

import ml_dtypes

import concourse.bass as bass
import concourse.tile as tile
from concourse import bacc, mybir
from concourse.bass_utils import run_bass_kernel_spmd
from concourse.masks import make_identity

VOCAB, E, F, KW, Y = 51917, 100, 50, 10, 8921
B, T = 8, 2500
PAD = 5
TP = T + 1          # conv output length 2501
TC = 20             # t-chunks of 128 (covers 2560)
TPADDED = TC * 128  # 2560
TLAST = TP - 19 * 128  # valid rows in last t-chunk = 69
GLAST = T - 19 * 128   # valid rows in last gather chunk = 68
ET_W = 2570            # conv reads up to 2048+9+512; zero-padded tail
YTILE = 512
NYT = 18            # y tiles; last one is YLAST wide
YLAST = Y - (NYT - 1) * YTILE + 1  # 218 (even: fp32r matmul ISA restriction)
YPAD = NYT * YTILE  # 9216
CONV_NT = 5         # conv t'-tiles of 512
HEXTW = 68          # h_ext block: 50 h cols, zeros, ones col at DEN
DEN = 64            # denominator column / acc row
KP = 25             # fp8 DoubleRow k-phys (F = 2*25)
FP32 = mybir.dt.float32
I32 = mybir.dt.int32
BF16 = mybir.dt.bfloat16
FP8 = mybir.dt.float8e4

# exp groups: (chunk_start, n_chunks, pool) alternating B(<=3 banks)/A(<=4)
GROUPS = [(0, 3, "B"), (3, 4, "A"), (7, 3, "B"), (10, 3, "A"), (13, 3, "B"),
          (16, 4, "A")]


def build_program(mm_kind="f32r", repeat=1):
    nc = bacc.Bacc(
        "TRN2",
        target_bir_lowering=False,
        debug=False,
        num_devices=B,
    )

    MF = mybir.dt.float32r if mm_kind == "f32r" else FP32
    DR = mybir.MatmulPerfMode.DoubleRow

    x_d = nc.dram_tensor("x_idx", [128, TC], I32, kind="ExternalInput")
    emb_d = nc.dram_tensor("w_emb", [VOCAB, E], FP32, kind="ExternalInput")
    cwt_d = nc.dram_tensor("cwt", [E, KW * F], MF, kind="ExternalInput")
    cb_d = nc.dram_tensor("conv_b", [F, 1], FP32, kind="ExternalInput")
    uw8_d = nc.dram_tensor("uw8", [KP, 2, YPAD], FP8, kind="ExternalInput")
    fwt_d = nc.dram_tensor("fwt", [F, YPAD], FP32, kind="ExternalInput")
    fb_d = nc.dram_tensor("fb", [NYT, YTILE], FP32, kind="ExternalInput")
    y_d = nc.dram_tensor("y", [NYT, YTILE], FP32, kind="ExternalOutput")

    EXP = mybir.ActivationFunctionType.Exp
    TANH = mybir.ActivationFunctionType.Tanh

    U32 = mybir.dt.uint32

    def ms0(ap):
        nc.vector.memset(ap.bitcast(U32), 0)

    def ms1(ap):
        nc.vector.memset(ap.bitcast(U32), 0x3F800000)

    with tile.TileContext(nc) as tc:
      for _rep in range(repeat):
        with tc.tile_pool(name="singles", bufs=1) as singles:
            identity = singles.tile([128, 128], FP32)
            make_identity(nc, identity[:])
            ones_col = singles.tile([F, 2], MF)
            ms1(ones_col[:])
            cb_sb = singles.tile([F, 1], FP32)
            nc.sync.dma_start(cb_sb[:], cb_d[:])
            cwt_sb = singles.tile([E, KW * F], MF)
            nc.sync.dma_start(cwt_sb[:], cwt_d[:])
            x_sb = singles.tile([128, TC], I32)
            nc.sync.dma_start(x_sb[:], x_d[:])
            fb_sb = singles.tile([NYT, YTILE], FP32)
            nc.sync.dma_start(fb_sb[:], fb_d[:])

            eT = singles.tile([E, ET_W], MF)
            ms0(eT[:])
            hT = singles.tile([F, TPADDED], FP32)
            hT8 = singles.tile([F, TPADDED], FP8)
            hT_r2 = singles.tile([KP, TC, 2, 128], FP8)
            h_ext = singles.tile([128, TC, HEXTW], MF)
            ms0(h_ext[:])
            numer_all = singles.tile([NYT, YTILE], FP32)
            nc.vector.memset(numer_all[:], 0.0)
            denom_all = singles.tile([NYT, YTILE], FP32)
            nc.vector.memset(denom_all[:], 1.0)

            # ---------------- setup: gather + conv + h_ext ----------------
            with (
                tc.tile_pool(name="gat", bufs=3) as gat,
                tc.tile_pool(name="ps_setup", bufs=2, space="PSUM") as ps_setup,
            ):
                # embedding gather: one offset per partition per chunk
                for c in range(TC):
                    rows = 128 if c < TC - 1 else GLAST
                    er = gat.tile([128, E], FP32, tag="er")
                    nc.gpsimd.indirect_dma_start(
                        out=er[0:rows, :],
                        out_offset=None,
                        in_=emb_d[:, :],
                        in_offset=bass.IndirectOffsetOnAxis(
                            ap=x_sb[0:rows, c : c + 1], axis=0
                        ),
                    )
                    pt = ps_setup.tile([128, 128], FP32, tag="ps")
                    nc.tensor.transpose(
                        pt[0:E, 0:rows], er[0:rows, :], identity[0:rows, 0:rows]
                    )
                    nc.vector.tensor_copy(
                        eT[:, PAD + 128 * c : PAD + 128 * c + rows], pt[0:E, 0:rows]
                    )

                # conv1d: hT[f, t'] = sum_k cwt[:, k].T @ eT[:, t'+k]
                # after each tile: tanh -> hT (fp32), fp8 copy -> hT8,
                # DMAs place the two 25-row K-tiles into hT_r2, and the
                # chunk transposes build h_ext (keeps PE order interleaved)
                for j in range(CONV_NT):
                    pc = ps_setup.tile([128, 512], FP32, tag="psc")
                    for k in range(KW):
                        nc.tensor.matmul(
                            pc[0:F, :],
                            cwt_sb[:, k * F : (k + 1) * F],
                            eT[:, j * 512 + k : j * 512 + k + 512],
                            start=(k == 0),
                            stop=(k == KW - 1),
                        )
                    nc.scalar.activation(
                        hT[:, j * 512 : (j + 1) * 512],
                        pc[0:F, :],
                        TANH,
                        bias=cb_sb[:],
                    )
                    nc.vector.tensor_copy(
                        hT8[:, j * 512 : (j + 1) * 512],
                        hT[:, j * 512 : (j + 1) * 512],
                    )
                    nc.sync.dma_start(
                        hT_r2[:, 4 * j : 4 * j + 4, 0, :],
                        hT8[0:KP, j * 512 : (j + 1) * 512],
                    )
                    nc.sync.dma_start(
                        hT_r2[:, 4 * j : 4 * j + 4, 1, :],
                        hT8[KP:F, j * 512 : (j + 1) * 512],
                    )
                    for c in range(4 * j, 4 * j + 4):
                        tv = 128 if c < TC - 1 else TLAST
                        ph = ps_setup.tile([128, 128], FP32, tag="ps")
                        nc.tensor.transpose(
                            ph[0:tv, 0:F],
                            hT[:, c * 128 : c * 128 + tv],
                            identity[0:F, 0:F],
                        )
                        nc.vector.tensor_copy(
                            h_ext[0:tv, c, 0:F], ph[0:tv, 0:F]
                        )
                        ms1(h_ext[0:tv, c, DEN : DEN + 1])

            # ---------------- main loop over y tiles ----------------
            with (
                tc.tile_pool(name="ps_acc", bufs=1, space="PSUM", side="right")
                as ps_acc,
                tc.tile_pool(name="ps_sB", bufs=1, space="PSUM", side="right")
                as ps_sB,
                tc.tile_pool(name="ps_sA", bufs=1, space="PSUM") as ps_sA,
                tc.tile_pool(name="uwp", bufs=3) as uwp,
                tc.tile_pool(name="fwp", bufs=2) as fwp,
                tc.tile_pool(name="expp", bufs=2) as expp,
                tc.tile_pool(name="finp", bufs=2) as finp,
            ):
                pending_fin = None
                for yt in range(NYT):
                    ys = yt * YTILE
                    W = YTILE if yt < NYT - 1 else YLAST
                    uw_t = uwp.tile([KP, 2, YTILE], FP8, tag="uw")
                    nc.sync.dma_start(uw_t[:, :, 0:W], uw8_d[:, :, ys : ys + W])
                    fw_t = fwp.tile([F, YTILE], FP32, tag="fw")
                    nc.sync.dma_start(fw_t[:, 0:W], fwt_d[:, ys : ys + W])

                    exps = expp.tile([128, TC, YTILE], MF, tag="exps")
                    acc = ps_acc.tile([HEXTW, YTILE], FP32, tag="acc")

                    def scores(gi, exps=exps, uw_t=uw_t, W=W):
                        c0, ng, pk = GROUPS[gi]
                        pool = ps_sA if pk == "A" else ps_sB
                        nb = 4 if pk == "A" else 3
                        ps = pool.tile([128, nb, YTILE], FP32, tag="s" + pk)
                        for i in range(ng):
                            nc.tensor.matmul(
                                ps[:, i, 0:W],
                                hT_r2[:, c0 + i],
                                uw_t[:, :, 0:W],
                                start=True,
                                stop=True,
                                perf_mode=DR,
                            )
                        return ps

                    def poolings(gi, exps=exps, acc=acc, W=W):
                        c0, ng, _ = GROUPS[gi]
                        for i in range(ng):
                            c = c0 + i
                            nc.tensor.matmul(
                                acc[:, 0:W],
                                h_ext[:, c],
                                exps[:, c, 0:W],
                                start=(c == 0),
                                stop=(c == TC - 1),
                            )

                    ps_cur = scores(0)
                    if pending_fin is not None:
                        pending_fin()
                        pending_fin = None
                    for gi in range(len(GROUPS)):
                        c0, ng, _ = GROUPS[gi]
                        nc.scalar.activation(
                            exps[:, c0 : c0 + ng, 0:W], ps_cur[:, 0:ng, 0:W], EXP
                        )
                        if gi + 1 < len(GROUPS):
                            ps_cur = scores(gi + 1)
                        poolings(gi)

                    def finalize(yt=yt, acc=acc, fw_t=fw_t, W=W):
                        # numer = ones.T @ (acc[0:F] * fwT); denom = acc[DEN]
                        p_t = finp.tile([F, YTILE], MF, tag="p")
                        nc.vector.tensor_mul(p_t[:, 0:W], acc[0:F, 0:W], fw_t[:, 0:W])
                        strip = finp.tile([128, YTILE], FP32, tag="strip")
                        nc.vector.tensor_copy(
                            strip[DEN : DEN + 1, 0:W], acc[DEN : DEN + 1, 0:W]
                        )
                        cs = ps_acc.tile([HEXTW, YTILE], FP32, tag="acc")
                        nc.tensor.matmul(
                            cs[0:2, 0:W], ones_col[:], p_t[:, 0:W],
                            start=True, stop=True,
                        )
                        nc.vector.tensor_copy(strip[0:1, 0:W], cs[0:1, 0:W])
                        nc.sync.dma_start(
                            numer_all[yt : yt + 1, 0:W], strip[0:1, 0:W]
                        )
                        nc.sync.dma_start(
                            denom_all[yt : yt + 1, 0:W], strip[DEN : DEN + 1, 0:W]
                        )

                    pending_fin = finalize

                pending_fin()

                # epilogue: y = numer / denom + fb
                recip = singles.tile([NYT, YTILE], FP32)
                nc.vector.reciprocal(recip[:], denom_all[:])
                yv = singles.tile([NYT, YTILE], FP32)
                nc.vector.tensor_mul(yv[:], numer_all[:], recip[:])
                yout = singles.tile([NYT, YTILE], FP32)
                nc.vector.tensor_add(yout[:], yv[:], fb_sb[:])
                nc.sync.dma_start(y_d[:, :], yout[:])

    nc.compile()
    return nc


_CACHE = {}

MM_KIND = "f32r"


def get_program(mm_kind=None, repeat=1):
    if mm_kind is None:
        mm_kind = MM_KIND
    key = (mm_kind, repeat)
    if key not in _CACHE:
        _CACHE[key] = build_program(mm_kind, repeat)
    return _CACHE[key]


def make_in_maps(x, W_emb, conv_w, conv_b, U_w, final_w, final_b):
    x = np.asarray(x).astype(np.int32)
    x_pad = np.zeros((B, TPADDED), np.int32)
    x_pad[:, :T] = x
    # x_maps[b][p, c] = x[b, c*128 + p]
    x_maps = np.ascontiguousarray(x_pad.reshape(B, TC, 128).transpose(0, 2, 1))

    emb = np.ascontiguousarray(np.asarray(W_emb, np.float32))
    # cwt[e, k*F + f] = conv_w[f, e, k]
    cwt = np.ascontiguousarray(
        np.asarray(conv_w, np.float32).transpose(1, 2, 0).reshape(E, KW * F)
    )
    cb = np.ascontiguousarray(np.asarray(conv_b, np.float32).reshape(F, 1))

    uwT = np.asarray(U_w, np.float32).T  # [F, Y]
    uw8 = np.zeros((KP, 2, YPAD), ml_dtypes.float8_e4m3)
    uw8[:, 0, :Y] = uwT[0:KP].astype(ml_dtypes.float8_e4m3)
    uw8[:, 1, :Y] = uwT[KP:F].astype(ml_dtypes.float8_e4m3)

    fwt = np.zeros((F, YPAD), np.float32)
    fwt[:, :Y] = np.asarray(final_w, np.float32).T

    fb = np.zeros((NYT, YTILE), np.float32)
    fb.reshape(-1)[:Y] = np.asarray(final_b, np.float32)

    return [
        dict(
            x_idx=x_maps[b],
            w_emb=emb,
            cwt=cwt,
            conv_b=cb,
            uw8=uw8,
            fwt=fwt,
            fb=fb,
        )
        for b in range(B)
    ]


def run(in_maps, trace=False, **kwargs):
    nc = get_program()
    return run_bass_kernel_spmd(
        nc, in_maps, core_ids=list(range(B)), trace=trace, **kwargs
    )


def kernel(x, W_emb, conv_w, conv_b, U_w, final_w, final_b):
    in_maps = make_in_maps(x, W_emb, conv_w, conv_b, U_w, final_w, final_b)
    res = run(in_maps)
    out = np.stack(
        [res.results[b]["y"].reshape(-1)[:Y] for b in range(B)]
    ).astype(np.float32)
    return out
